# revision 23
# baseline (speedup 1.0000x reference)
"""GAT layer (PyG GATConv, concat=False, edge_dim=1) on 8 Trainium2 cores.

Sharding: dst-node 128-tiles LPT-assigned to cores (10 slots each); the graph
is batch-independent so each gathered row carries all 4 batches.

v2 design (vs v1): attention logits alpha = a_src[src]+a_dst[dst]+attr*c are
linear in x, so the host precomputes them (like w_ext/indtab prep).  This
shrinks gathered rows to h-only 2048 B (exact, no pad), kills the a_dst
indicator-expansion matmuls + indT tables, and lets phase-2 start as a pure
gather+scatter pipeline.  The SWDGE gather descriptors are generated with
prepare_only=True so the ~9.5us/chunk Q7 desc-gen runs on GpSimd UNDER
phase 1 (8 chunks ahead = ga pool depth), each chunk fired later with
trigger_dma(count=None); Tile defers the tableH RAW edge to the trigger.

Per core:
  phase 1: h[b] = x[b] @ W(o,h) for all N nodes; xT streamed per node-tile
    ([P, B*P] tiles), PSUM [P, B, 256] (2 banks), one fused [m, 1024] copy
    per tile alternating Scalar/Vector, tableH rows [4 x 256 (o,h)] fp16
    written in 4-tile groups.
  phase 2: edges sorted by dst tile, 128-edge blocks, 8 blocks per chunk.
    trigger -> gather rows; alpha loaded per chunk [P, 8*16] fp16;
    lr = leakyrelu on DVE, p = exp(lr-4) on ScalarE (softmax-invariant
    shift keeping fp16 exp in range).  phg = p*h with (b,o,h) broadcast;
    PSUM accumulation accn += Ind.T @ phg, accd += Ind.T @ p; epilogue
    per dst tile: divide, mean over heads, + bias, per-tile y write.
"""

import numpy as np
import ml_dtypes

B, N, E, D, H, O = 4, 10000, 160000, 128, 4, 64
NEG_SLOPE = 0.2
P = 128
HO = H * O                        # 256
NTILE = -(-N // P)                # 79 global dst tiles
NT = 10                           # dst tile slots per core
NROWT = NTILE * P                 # 10112 table rows
ROW = B * HO                      # 1024 fp16 elems per tableH row (2048 B)
FW = B * HO                       # 1024
BH = B * H                        # 16
CHUNK = 8                         # blocks per gather call (1024 edges)
NCORE = 8
EXP_SHIFT = 4.0                   # exp(x - shift): softmax-invariant
WG = 4                            # tiles per tableH write
GA_BUFS = 8                       # gather buffers = prepared chunks ahead

_cache = {}


def _build_program(meta):
    import concourse.bacc as bacc
    import concourse.mybir as mybir
    from concourse.tile import TileContext

    f32 = mybir.dt.float32
    f16 = mybir.dt.float16
    i16 = mybir.dt.int16
    Alu = mybir.AluOpType
    Act = mybir.ActivationFunctionType

    nblk = meta["nblk"]
    blk_tile = meta["blk_tile"]
    blk_first = meta["blk_first"]
    blk_last = meta["blk_last"]
    ne = nblk * P
    nch = ne // (CHUNK * P)

    nc = bacc.Bacc("TRN2", target_bir_lowering=False, debug=False,
                   num_devices=NCORE, num_swdge_queues=4)

    xTt = nc.dram_tensor("xTt", [NTILE, P, B * P], f16, kind="ExternalInput")
    w_oh = nc.dram_tensor("w_oh", [P, HO], f16, kind="ExternalInput")
    bias_bc = nc.dram_tensor("bias_bc", [P, B * O], f32, kind="ExternalInput")
    alph = nc.dram_tensor("alph", [P, nblk * BH], f16, kind="ExternalInput")
    indtab = nc.dram_tensor("indtab", [nch, P, CHUNK * P], f16,
                            kind="ExternalInput")
    idxA = nc.dram_tensor("idxA", [P, ne // 16], i16, kind="ExternalInput")
    y = nc.dram_tensor("y", [NT * P, B * O], f32, kind="ExternalOutput")

    tableH = nc.dram_tensor("tableH", [NROWT, ROW], f16, kind="Internal")

    from concourse.library_config import mlp

    with TileContext(nc) as tc:
        with (
            tc.tile_pool(name="persist", bufs=1) as pp,
            tc.tile_pool(name="ga", bufs=1) as gap,
        ):
            nc.gpsimd.load_library(mlp)

            bias_sb = pp.tile([P, B * O], f32)
            nc.sync.dma_start(out=bias_sb[:], in_=bias_bc[:])
            idxA_sb = pp.tile([P, ne // 16], i16)
            nc.sync.dma_start(out=idxA_sb[:], in_=idxA[:])
            shift_sb = pp.tile([P, 1], f32)
            nc.vector.memset(shift_sb[:], -EXP_SHIFT)

            dma_sem = nc.alloc_semaphore("ga_dma")

            # stable ga ring buffers, allocated before the phase-1 pools so
            # their SBUF zone never overlaps phase-1 tiles (an overlap gives
            # the preps WAR deps on late phase-1 instructions)
            ga_slots = [gap.tile([P, CHUNK, ROW], f16, tag=f"ga{j}",
                                 name=f"gabuf{j}")
                        for j in range(GA_BUFS)]
            ga_tiles = {}
            pending_q = [0]

            def emit_prep(ch):
                ga = ga_slots[ch % GA_BUFS]
                ga_tiles[ch] = ga
                nc.gpsimd.dma_gather(
                    ga[:], tableH.ap()[:, :],
                    idxA_sb[:, ch * 64:(ch + 1) * 64],
                    CHUNK * P, CHUNK * P, ROW,
                    prepare_only=True, sem=dma_sem,
                    single_packet=False, queue_num=0)
                pending_q[0] += 1

            # ---- phase 1 ----
            with (
                tc.tile_pool(name="p1x", bufs=1) as p1x,
                tc.tile_pool(name="p1xt", bufs=4) as p1xt,
                tc.tile_pool(name="p1h", bufs=3) as p1h,
                tc.tile_pool(name="psum_h", bufs=3, space="PSUM") as psh,
            ):
                woh_sb = p1x.tile([P, HO], f16, tag="woh")
                nc.sync.dma_start(out=woh_sb[:], in_=w_oh[:])
                XG = 8
                hst4 = None
                xtg = None
                for t in range(NTILE):
                    m = min(P, N - t * P)
                    tq = t % WG
                    if tq == 0:
                        ng = min(WG, NTILE - t)
                        hst4 = p1h.tile([P, WG, ROW], f16, tag="hst")
                    if t % XG == 0:
                        nx = min(XG, NTILE - t)
                        xtg = p1xt.tile([P, XG, B * P], f16, tag="xtt")
                        nc.sync.dma_start(
                            out=xtg[:, 0:nx, :],
                            in_=xTt.ap()[t:t + nx].rearrange(
                                "g p c -> p g c"))
                    xtt = xtg[:, t % XG, :]
                    # two single-bank PSUM tiles (bank-crossing APs are slow)
                    hp0 = psh.tile([P, 2, HO], f32, space="PSUM", tag="hp0")
                    hp1 = psh.tile([P, 2, HO], f32, space="PSUM", tag="hp1")
                    for b in range(B):
                        hp = hp0 if b < 2 else hp1
                        nc.tensor.matmul(hp[:m, b % 2, :],
                                         lhsT=xtt[b * P:(b + 1) * P][:, 0:m]
                                         if False else
                                         xtt[:, b * P:b * P + m],
                                         rhs=woh_sb[:],
                                         start=True, stop=True)
                    if m < P:
                        nc.vector.memset(hst4[:, tq, :], 0.0)
                    d0 = hst4[:m, tq, 0:2 * HO].rearrange(
                        "p (b c) -> p b c", b=2)
                    d1 = hst4[:m, tq, 2 * HO:].rearrange(
                        "p (b c) -> p b c", b=2)
                    nc.scalar.copy(d0, hp0[:m, :, :])
                    nc.vector.tensor_copy(d1, hp1[:m, :, :])
                    if tq == ng - 1:
                        t0 = t - tq
                        nc.sync.dma_start(
                            out=tableH.ap()[t0 * P:(t0 + ng) * P, :]
                            .rearrange("(q p) c -> p q c", p=P),
                            in_=hst4[:, 0:ng, :])

            # Emitted after the tableH writes so the deferred RAW edge binds
            # to the triggers; GpSimd still runs this desc-gen under phase 1
            # (engine streams are independent and preps have no unmet deps).
            for ch in range(min(GA_BUFS, nch)):
                emit_prep(ch)

            # ---- phase 2 ----
            with (
                tc.tile_pool(name="rr", bufs=3) as rrp,
                tc.tile_pool(name="al", bufs=3) as alp,
                tc.tile_pool(name="wk", bufs=3) as wp,
                tc.tile_pool(name="bk", bufs=4) as bp,
                tc.tile_pool(name="psum_num", bufs=3, space="PSUM") as psn,
                tc.tile_pool(name="psum_den", bufs=2, space="PSUM") as psd,
            ):
                for ch in range(nch):
                    # count=None fires every untriggered prep (Tile-managed
                    # gating); skip when nothing is pending — those chunks'
                    # gathers were already fired by an earlier trigger.
                    if pending_q[0] > 0:
                        nc.gpsimd.trigger_dma(count=None, queue_num=0)
                        pending_q[0] = 0
                    ga = ga_tiles[ch]

                    alpha_t = alp.tile([P, CHUNK * BH], f16, tag="alpha")
                    nc.sync.dma_start(
                        out=alpha_t[:],
                        in_=alph.ap()[:, ch * CHUNK * BH:(ch + 1) * CHUNK * BH])
                    ind_sb = rrp.tile([P, CHUNK * P], f16, tag="inds")
                    nc.sync.dma_start(out=ind_sb[:], in_=indtab.ap()[ch])

                    CW = CHUNK * BH               # 128
                    # leaky relu fused: lr = max(0.2*a, a)
                    lr = wp.tile([P, CW], f32, tag="lr")
                    nc.vector.scalar_tensor_tensor(
                        lr[:], alpha_t[:], NEG_SLOPE, alpha_t[:],
                        Alu.mult, Alu.max)
                    # p = exp(lr - EXP_SHIFT) in fp16
                    pall = wp.tile([P, CW], f16, tag="pall")
                    nc.scalar.activation(pall[:], lr[:], Act.Exp,
                                         bias=shift_sb[:])

                    for b8 in range(CHUNK):
                        blk = ch * CHUNK + b8
                        t = blk_tile[blk]
                        px = pall[:, b8 * BH:(b8 + 1) * BH]
                        # phg = p * h ((b,o,h): broadcast middle axis)
                        phg = bp.tile([P, FW], f16, tag="phg")
                        nc.vector.tensor_tensor(
                            phg[:].rearrange("p (b o h) -> p b o h",
                                             b=B, o=O),
                            ga[:, b8, :].rearrange(
                                "p (b o h) -> p b o h", b=B, o=O),
                            px.rearrange("p (b h) -> p b h", b=B)
                            .unsqueeze(2).broadcast_to([P, B, O, H]),
                            Alu.mult)
                        if blk_first[blk]:
                            accn = psn.tile([P, FW], f32, space="PSUM",
                                            tag="an")
                            accd = psd.tile([P, BH], f32, space="PSUM",
                                            tag="ad")
                            meta["psum_tiles"][t] = (accn, accd)
                        accn, accd = meta["psum_tiles"][t]
                        nc.tensor.matmul(accn[:, 0:FW // 2],
                                         lhsT=ind_sb[:, b8 * P:(b8 + 1) * P],
                                         rhs=phg[:, 0:FW // 2],
                                         start=blk_first[blk],
                                         stop=blk_last[blk],
                                         skip_group_check=True)
                        nc.tensor.matmul(accn[:, FW // 2:],
                                         lhsT=ind_sb[:, b8 * P:(b8 + 1) * P],
                                         rhs=phg[:, FW // 2:],
                                         start=blk_first[blk],
                                         stop=blk_last[blk],
                                         skip_group_check=True)
                        nc.tensor.matmul(
                            accd[:], lhsT=ind_sb[:, b8 * P:(b8 + 1) * P],
                            rhs=px,
                            start=blk_first[blk], stop=blk_last[blk],
                            skip_group_check=True)

                        if blk_last[blk]:
                            den = bp.tile([P, BH], f32, tag="den")
                            nc.vector.tensor_scalar(den[:], accd[:], 1e-16,
                                                    None, Alu.max)
                            rec = bp.tile([P, BH], f32, tag="rec")
                            nc.vector.reciprocal(rec[:], den[:])
                            onum = bp.tile([P, FW], f32, tag="onum")
                            nc.vector.tensor_tensor(
                                onum[:].rearrange("p (b o h) -> p b o h",
                                                  b=B, o=O),
                                accn[:].rearrange("p (b o h) -> p b o h",
                                                  b=B, o=O),
                                rec[:].rearrange("p (b h) -> p b h", b=B)
                                .unsqueeze(2).broadcast_to([P, B, O, H]),
                                Alu.mult)
                            hsum = bp.tile([P, B * O], f32, tag="hsum")
                            nc.vector.tensor_reduce(
                                hsum[:].rearrange("p (b o) -> p b o", b=B),
                                onum[:].rearrange("p (b o h) -> p b o h",
                                                  b=B, o=O),
                                axis=mybir.AxisListType.X, op=Alu.add)
                            out_t = bp.tile([P, B * O], f32, tag="outt")
                            nc.vector.scalar_tensor_tensor(
                                out_t[:], hsum[:], 1.0 / H, bias_sb[:],
                                Alu.mult, Alu.add)
                            nc.sync.dma_start(
                                out=y.ap()[t * P:(t + 1) * P, :],
                                in_=out_t[:])

                    if ch + GA_BUFS < nch:
                        emit_prep(ch + GA_BUFS)

    _fix_prep_sems(nc, mybir)
    nc.compile()
    return nc


def _fix_prep_sems(nc, mybir):
    """Point each prepare_only gather's DMA-completion sem (OnUpdate[0]) at
    the Tile-assigned DMASW lane sem for its ring position.

    Tile assigns SWDGE DMAs round-robin DMASW0..7 lanes and generates all
    consumer waits / slot-reuse doorbell guards against those lane sems, but
    the prepare_only API bakes the caller's sem into the descriptor, so the
    lane sems would never be incremented.  Rewriting the prep's update to
    lane (k % 8, +16) makes the whole Tile-generated protocol consistent:
    consumers of chunk k wait lane k%8 >= 16*(k//8+1), and the doorbell
    guards serialize lane reuse exactly as Tile intended.
    """
    import re
    lane_ids = {}
    insts = [i for blk in nc.m.functions[0].blocks for i in blk.instructions]
    for ins in insts:
        si = ins.sync_info
        if not si:
            continue
        for w in list(si.on_wait) + list(si.on_update):
            mm = re.match(r"DMASW(\d)_", w.ant_name or "")
            if mm:
                lane_ids[int(mm.group(1))] = (w.id, w.ant_name)
    preps = [i for i in insts
             if type(i).__name__ == "InstDMAGatherAnt"
             and getattr(i, "gen_mode", 0) == 1]
    for k, prep in enumerate(preps):
        lane = k % 8
        if lane not in lane_ids:
            continue
        upd = prep.sync_info.on_update[0]
        assert upd.ant_name and "ga_dma" in upd.ant_name, upd
        sem_id, sem_name = lane_ids[lane]
        prep.sync_info.on_update[0] = mybir.SyncUpdate(
            sync_type="semaphore", id=sem_id,
            ant_name=sem_name, update_mode="sem-add-imm",
            update_value=16, update_reg=None)

    # Neutralize Tile's per-prep lane pre-bumps (IncSwdgeSem add+16): with
    # the completion rewrite above, lane sems must move ONLY on true DMA
    # completion, else consumers un-gate at prep time.  Their lane-reuse
    # waits are redundant: the prep's ga-buffer WAR wait (consumers of the
    # previous occupant) already implies the previous same-lane gather
    # completed, and the trigger follows the prep in Pool program order.
    # Move non-deferred tableH RAW waits (DMAHW lanes) off the preps onto
    # the first trigger: desc-gen reads only the idx metadata; the table is
    # read by the DMA, which fires at the trigger.  Replace them on the prep
    # with first-DMA-per-lane waits (>=16) so the early idx/bias/w loads
    # still gate desc-gen.
    triggers = [i for i in insts if type(i).__name__ == "InstTriggerDma"]
    moved = {}
    for prep in preps:
        si = prep.sync_info
        keep, lanes_seen = [], set()
        for w in si.on_wait:
            nm = w.ant_name or ""
            if nm.startswith("DMAHW") and w.wait_value > 16:
                key = (w.id, nm)
                moved[key] = max(moved.get(key, 0), w.wait_value)
                lanes_seen.add(key)
            else:
                keep.append(w)
        for (sid2, nm2) in lanes_seen:
            keep.append(mybir.SyncWait(
                sync_type="semaphore", id=sid2, ant_name=nm2,
                wait_mode="sem-ge-imm", wait_value=16, wait_reg=None))
        si.on_wait = keep
    if triggers and moved:
        tsi = triggers[0].sync_info
        for (sid2, nm2), val in moved.items():
            tsi.on_wait.append(mybir.SyncWait(
                sync_type="semaphore", id=sid2, ant_name=nm2,
                wait_mode="sem-ge-imm", wait_value=val, wait_reg=None))

    lane_id_set = {sid for sid, _ in lane_ids.values()}
    for blk in nc.m.functions[0].blocks:
        keep = []
        for ins in blk.instructions:
            if (type(ins).__name__ == "InstIncSwdgeSem"
                    and getattr(ins, "_mode", "") == "add"
                    and getattr(ins, "_sem_id_base", -1) in lane_id_set
                    and list(getattr(ins, "_sem_values", [])) == [16]):
                continue
            keep.append(ins)
        if len(keep) != len(blk.instructions):
            blk.instructions = keep


def _preprocess(inputs):
    x = np.asarray(inputs["x"], np.float32)
    edge_index = np.asarray(inputs["edge_index"])
    edge_attr = np.asarray(inputs["edge_attr"], np.float32)
    W_src = np.asarray(inputs["W_src"], np.float32)
    att_src = np.asarray(inputs["att_src"], np.float32)
    att_dst = np.asarray(inputs["att_dst"], np.float32)
    W_edge = np.asarray(inputs["W_edge"], np.float32)
    att_edge = np.asarray(inputs["att_edge"], np.float32)
    bias = np.asarray(inputs["bias"], np.float32)

    src = edge_index[0].astype(np.int64)
    dst = edge_index[1].astype(np.int64)

    # weights: h columns in (o, head) order so broadcasts have stride-1 heads
    W_oh16 = np.ascontiguousarray(
        W_src.transpose(0, 2, 1).reshape(D, HO)).astype(np.float16)
    # attention logits per edge, computed on host (linear in x)
    Wa_src = np.einsum("dho,ho->dh", W_src, att_src)
    Wa_dst = np.einsum("dho,ho->dh", W_src, att_dst)
    a_src = np.einsum("bnd,dh->bnh", x, Wa_src)          # [B,N,H]
    a_dst = np.einsum("bnd,dh->bnh", x, Wa_dst)
    c = np.einsum("ho,ho->h", W_edge, att_edge)          # [H]
    alpha_full = (a_src[:, src, :] + a_dst[:, dst, :]
                  + edge_attr[None, :, None] * c[None, None, :])  # [B,E,H]
    alpha_full = alpha_full.astype(np.float16)

    bias_bc = np.tile(bias, B)[None, :].repeat(P, 0).copy()

    # xTt[t, d, b*P+j] = x[b, t*P+j, d], zero-padded past N
    xpad = np.zeros((B, NTILE * P, D), np.float16)
    xpad[:, 0:N, :] = x.astype(np.float16)
    xTt = np.ascontiguousarray(
        xpad.reshape(B, NTILE, P, D).transpose(1, 3, 0, 2).reshape(
            NTILE, P, B * P))

    # LPT assignment of global dst tiles to cores (balance edge counts)
    gtile = dst // P
    cnt_g = np.bincount(gtile, minlength=NTILE)
    order = np.argsort(-cnt_g, kind="stable")
    core_tot = np.zeros(NCORE, np.int64)
    core_tiles = [[] for _ in range(NCORE)]
    for g in order:
        cand = sorted(range(NCORE),
                      key=lambda cc: (core_tot[cc], len(core_tiles[cc])))
        for cc in cand:
            if len(core_tiles[cc]) < NT:
                core_tiles[cc].append(g)
                core_tot[cc] += cnt_g[g]
                break
    g_map = np.full((NCORE, NT), -1, np.int64)
    cnt_s = np.zeros((NCORE, NT), np.int64)
    for cc in range(NCORE):
        ts = sorted(core_tiles[cc], key=lambda g: -cnt_g[g])
        for j, g in enumerate(ts):
            g_map[cc, j] = g
            cnt_s[cc, j] = cnt_g[g]

    bt = np.maximum(1, -(-cnt_s.max(axis=0) // P))
    total = int(bt.sum())
    bt[NT - 1] += -(-total // CHUNK) * CHUNK - total
    nblk = int(bt.sum())
    ne = nblk * P
    starts = np.concatenate([[0], np.cumsum(bt)])

    blk_tile = np.repeat(np.arange(NT), bt)
    blk_first = np.zeros(nblk, bool)
    blk_last = np.zeros(nblk, bool)
    blk_first[starts[:-1]] = True
    blk_last[starts[1:] - 1] = True

    meta = {"nblk": nblk, "blk_tile": blk_tile.tolist(),
            "blk_first": blk_first.tolist(), "blk_last": blk_last.tolist(),
            "psum_tiles": {}, "g_map": g_map}

    def wrap16(a, chunklen=1024):
        ncalls = len(a) // chunklen
        w = a.astype(np.int16).reshape(ncalls, chunklen // 16, 16)
        w = w.transpose(2, 0, 1).reshape(16, -1)
        return np.tile(w, (8, 1)).copy()

    in_maps = []
    for cc in range(NCORE):
        srcg = np.zeros(ne, np.int64)
        eid = np.full(ne, -1, np.int64)
        for j in range(NT):
            g = g_map[cc, j]
            if g < 0:
                continue
            sel = np.nonzero(gtile == g)[0]
            sl0 = starts[j] * P
            k = len(sel)
            srcg[sl0:sl0 + k] = src[sel]
            eid[sl0:sl0 + k] = sel

        # indicator tables [blk, e, n] -> [nch, P, CHUNK*P]
        reld = np.full(ne, -1.0, np.float32)
        for j in range(NT):
            g = g_map[cc, j]
            if g < 0:
                continue
            sel = np.nonzero(gtile == g)[0]
            sl0 = starts[j] * P
            reld[sl0:sl0 + len(sel)] = (dst[sel] - g * P).astype(np.float32)
        nch = ne // 1024
        rel_b = reld.reshape(nblk, P)
        ind_full = (rel_b[:, :, None] ==
                    np.arange(P)[None, None, :])            # [blk, e, n]
        indtab = np.ascontiguousarray(
            ind_full.transpose(1, 0, 2).reshape(P, nblk, P)
            .reshape(P, nch, CHUNK * P).transpose(1, 0, 2)
        ).astype(np.float16)

        # alpha per edge-slot: [P, nblk*BH], (b, h) cols, 0 for dummies
        esafe = np.maximum(eid, 0)
        al = alpha_full[:, esafe, :]                        # [B, ne, H]
        al = al * (eid >= 0)[None, :, None].astype(np.float16)
        alc = np.ascontiguousarray(
            al.reshape(B, nblk, P, H).transpose(2, 1, 0, 3)
            .reshape(P, nblk * BH))

        m = {
            "idxA": wrap16(srcg),
            "indtab": indtab,
            "alph": alc,
            "xTt": xTt,
            "w_oh": W_oh16,
            "bias_bc": bias_bc.astype(np.float32),
        }
        in_maps.append(m)
    return meta, in_maps


def _unshard(meta, results):
    g_map = meta["g_map"]
    out = np.empty((B, N, O), np.float32)
    for cc in range(NCORE):
        yc = results[cc]["y"]                 # [1280, 256]
        for j in range(NT):
            g = g_map[cc, j]
            if g < 0:
                continue
            m = min(P, N - g * P)
            for b in range(B):
                out[b, g * P:g * P + m, :] = \
                    yc[j * P:j * P + m, b * O:(b + 1) * O]
    return out


def kernel(**inputs):
    from concourse.bass_utils import run_bass_kernel_spmd

    meta, in_maps = _preprocess(inputs)
    key = meta["nblk"]
    if key not in _cache:
        _cache[key] = _build_program(meta)
    nc = _cache[key]

    res = run_bass_kernel_spmd(nc, in_maps, core_ids=list(range(NCORE)))
    return _unshard(meta, res.results)
